# revision 29
# baseline (speedup 1.0000x reference)
"""Bidirectional H=1 LSTM attention kernel for Trainium2 (8 NeuronCores), v4.

Model: hs = BiLSTM(x) [B,T,2] -> att = softmax(mean(hs,-1), axis=T) -> out = att[:,:,None]*x
Shapes: B=32, T=4096, E=300, H=1.

v4 changes vs v2 (141us) / v3 (146us):
  - the DRAM dxg round trip is gone: PSUM drains write a resident fp16
    xball [8, BL*(W+T+W)] (rows (d,g), per-batch zero-padded t columns),
    and the scan layout is gathered with single-partition SBUF->SBUF DMAs
    whose waits are engine sems (sub-us) instead of HBM write receipts.
    Phase-1 HBM traffic drops from 12.5MB to 9.9MB per core.
  - xg_tile/btmp are fp16 (validated offline: rel err unchanged at
    2.18e-3); the scan STT reads the fp16 xg directly.
  - gathers for batches 0-1 issue mid-phase-1 (top of chunk 6, when their
    drains are long done, so the DMA waits don't block the queues);
    batches 2-3 + reverse are the only tail work after the last matmul.
  - phase 5: all multiplies on VE at 4096 cols (v3 showed GpSimd TT is
    ~4x slower AND concurrent GP+VE degrade each other ~4x via SBUF port
    contention); broadcast PSUM tiles [128,1024]x4 for a finer PE->ACT
    pipeline.
  - the Exp ACT table load (~1.3us) is prefetched during the last scan
    iteration via a dummy 1-element Exp.
"""

import sys

sys.path.insert(0, "/opt/trn_rl_repo")

import numpy as np
from contextlib import ExitStack

import concourse.bass as bass
import concourse.bacc as bacc
import concourse.tile as tile
from concourse import mybir
from concourse.bass_utils import run_bass_kernel_spmd

F32 = mybir.dt.float32
F16 = mybir.dt.float16
AF = mybir.ActivationFunctionType
ALU = mybir.AluOpType

NCORES = 8
B, T, E = 32, 4096, 300
BL = B // NCORES          # batches per core
TOK = BL * T              # tokens per core (b-major)
L, W = 256, 32            # chunk len, halo warmup
S = L + W                 # scan steps per chunk
K = T // L                # chunks per (dir, batch)
P = 2 * BL * K            # partitions = d*64 + b*16 + k = 128
N_ITER = 3                # fixed-point iterations (2 fails: 2.18e-2)
XROW = W + T + W          # padded per-batch row: W zeros, T data, W zeros
XCOLS = BL * XROW         # xball columns
# gate order inside a block row: (i, f, o, g) ; pytorch order is (i, f, g, o)
GATE_PERM = [0, 1, 3, 2]


def _build_nc():
    nc = bacc.Bacc(None, target_bir_lowering=False, debug=False)
    xt0 = nc.declare_dram_parameter("xt0", [128, TOK], F16, isOutput=False)
    xt1 = nc.declare_dram_parameter("xt1", [128, TOK], F16, isOutput=False)
    xt2 = nc.declare_dram_parameter("xt2", [45, TOK], F16, isOutput=False)
    w8a_d = nc.declare_dram_parameter("w8a", [128, 8], F16, isOutput=False)
    w8b_d = nc.declare_dram_parameter("w8b", [128, 8], F16, isOutput=False)
    w8c_d = nc.declare_dram_parameter("w8c", [45, 8], F16, isOutput=False)
    whh = nc.declare_dram_parameter("whh", [P, 4], F32, isOutput=False)
    sel = nc.declare_dram_parameter("sel", [64, 4], F32, isOutput=False)
    selT = nc.declare_dram_parameter("selT", [4, 64], F32, isOutput=False)
    outT = nc.declare_dram_parameter("outT", [E, TOK], F16, isOutput=True)

    with tile.TileContext(nc) as tc, ExitStack() as ctx:
        singles = ctx.enter_context(tc.tile_pool(name="singles", bufs=1))
        scanctx = ExitStack()
        scanp = scanctx.enter_context(tc.tile_pool(name="scanp", bufs=1))
        xbctx = ExitStack()
        xbp = xbctx.enter_context(tc.tile_pool(name="xbp", bufs=1))
        p1ctx = ExitStack()
        psA = p1ctx.enter_context(tc.tile_pool(name="psA", bufs=6, space="PSUM"))
        psS = p1ctx.enter_context(tc.tile_pool(name="psS", bufs=1, space="PSUM"))

        # ---- resident tiles ----
        xt0_sb = singles.tile([128, TOK], F16)   # e 0..127 resident
        xt1_sb = singles.tile([128, TOK], F16)   # e 128..255 resident
        xt2_sb = singles.tile([45, TOK], F16)    # e 256..299 + ones row
        h_st = singles.tile([128, S + 1], F32)   # col 0 stays zero
        # scan-layout gather targets (fp16)
        xg_tile = scanp.tile([128, 4 * S], F16, tag="xg")
        btmp = scanp.tile([64, 4 * S], F16, tag="btmp")
        # xg staging: rows (d*4+g), cols b*XROW + (0..W zeros)(W..W+T data)
        xball = xbp.tile([8, XCOLS], F16, tag="xball")

        # ---- phase 1: stream x (fp16), compute xg -> xball -> xg_tile ----
        BOUNDS = [0, 1024, 2048, 4096, 6144, 8192, 10240, 12288, 14336,
                  16384]
        NCC = len(BOUNDS) - 1
        # weights first (tiny, needed by the first matmuls), then ALL x
        # loads issued upfront: the xt tiles are resident with no slot
        # reuse, so the HWDGE rings can stream the whole 9.9MB back to
        # back, decoupled from the compute pipeline (per-chunk prefetch
        # left the queues idling behind compute-gated issues).
        w8a = singles.tile([128, 8], F16)
        nc.scalar.dma_start(out=w8a, in_=w8a_d[:, :])
        w8b = singles.tile([128, 8], F16)
        nc.scalar.dma_start(out=w8b, in_=w8b_d[:, :])
        w8c = singles.tile([45, 8], F16)
        nc.scalar.dma_start(out=w8c, in_=w8c_d[:, :])
        # all loads issued upfront (resident tiles, no WAR), with load
        # chunks MATCHED to compute chunks: coarser load chunks make each
        # compute chunk wait longer than its own compute time and the
        # pipeline falls ~2us further behind per chunk.
        for lc in range(NCC):
            cols = slice(BOUNDS[lc], BOUNDS[lc + 1])
            nc.gpsimd.dma_start(out=xt0_sb[:, cols], in_=xt0[:, cols])
            nc.scalar.dma_start(out=xt1_sb[:, cols], in_=xt1[:, cols])
            nc.sync.dma_start(out=xt2_sb[:, cols], in_=xt2[:, cols])
        ones1 = singles.tile([1, 128], F32)
        nc.vector.memset(ones1[:, :], 1.0)
        ones16 = singles.tile([1, 128], F16)
        nc.vector.memset(ones16[:, :], 1.0)
        nc.vector.memset(h_st[:, :], 0.0)
        negone = singles.tile([64, 1], F32)
        nc.vector.memset(negone[:, :], -1.0)
        # preload the sigmoid/tanh ACT table set now (identity lives in
        # sigmoid_and_others too, so the phase-1 drains are unaffected and
        # the scan doesn't pay the ~1.3us table load on its critical path).
        sgscr = singles.tile([1, 1], F32)
        nc.scalar.activation(sgscr, ones1[0:1, 0:1], AF.Sigmoid)
        # zero the per-batch halo pads of xball
        for b in range(BL):
            nc.vector.memset(xball[:, b * XROW:b * XROW + W], 0.0)
            nc.vector.memset(xball[:, b * XROW + W + T:(b + 1) * XROW], 0.0)

        psscr = psS.tile([8, 1024], F32, tag="scr")
        # touch matmuls: 1-wait-each reads of freshly DMA'd tensors so the
        # PE clock passes every DMA before real matmuls issue.
        nc.tensor.matmul(psscr[0:8, 0:8], lhsT=w8a, rhs=w8a,
                         start=True, stop=True)
        nc.tensor.matmul(psscr[0:8, 8:16], lhsT=w8b, rhs=w8b,
                         start=True, stop=True)
        nc.tensor.matmul(psscr[0:8, 16:24], lhsT=w8c, rhs=w8c,
                         start=True, stop=True)
        # HAM warmup: ~5us of back-to-back dummy matmuls while the first
        # chunks stream in, so the PE clock gate reliably reaches 8/8 (the
        # HAM needs one fully-busy 3.4us SHORT window).  The rhs MUST be a
        # never-written scratch tile: reading an xt range gave the warmup a
        # RAW dep on that chunk's load, the scheduler pushed it to the END
        # of phase 1, and the whole matmul stream ran at K=4/8 half clock.
        wscratch = singles.tile([128, 512], F16)
        nc.vector.memset(wscratch[:, :], 0.0)
        for wu in range(12):
            nc.tensor.matmul(psscr[0:8, 512:1024],
                             lhsT=w8a, rhs=wscratch,
                             start=True, stop=True)
        whh_sb = singles.tile([P, 4], F32)
        nc.sync.dma_start(out=whh_sb, in_=whh[:, :])
        sel_sb = singles.tile([64, 4], F32)
        nc.sync.dma_start(out=sel_sb, in_=sel[:, :])
        selT_sb = singles.tile([4, 64], F32)
        nc.sync.dma_start(out=selT_sb, in_=selT[:, :])
        nc.tensor.matmul(psscr[0:4, 24:28], lhsT=sel_sb, rhs=sel_sb,
                         start=True, stop=True)
        nc.tensor.matmul(psscr[0:2, 28:30], lhsT=selT_sb[:, 0:2],
                         rhs=selT_sb[:, 0:2], start=True, stop=True)

        def emit_gathers(bb, tail=False):
            """Gather batch bb from xball into the scan layout.

            fwd (d=0) rows land in xg_tile directly; bwd (d=1) rows land in
            btmp (natural time order) for the later reversed copy.  One DMA
            per (d, g): src is one xball partition row with an overlapping
            halo AP (DMA APs max 3 dims incl the partition dim), dst is 16
            contiguous partitions.  The waits are VE/ACT drain sems, not
            HBM receipts, so these cost only issue time.
            """
            for d in range(2):
                for g in range(4):
                    src = bass.AP(
                        tensor=xball[:, :].tensor,
                        offset=(d * 4 + g) * XCOLS + bb * XROW
                        + (W if d == 1 else 0),
                        ap=[[XCOLS, 1], [L, K], [1, S]])
                    if d == 0:
                        dst = xg_tile[bb * 16:(bb + 1) * 16,
                                      g * S:(g + 1) * S]
                        # fwd on gpsimd (after all its load issues);
                        # on the tail spread over gpsimd+sync.
                        eng = (nc.sync if (tail and g >= 2) else nc.gpsimd)
                        eng.dma_start(out=dst, in_=src)
                    else:
                        dst = btmp[bb * 16:(bb + 1) * 16,
                                   g * S:(g + 1) * S]
                        # bwd on sync mid-phase (scalar is busy with ACT
                        # drains); on the tail spread scalar+gpsimd.
                        if tail:
                            eng = nc.scalar if g < 2 else nc.gpsimd
                        else:
                            eng = nc.sync
                        eng.dma_start(out=dst, in_=src)

        def emit_reverse(bp):
            """Reversed VE copy time-aligning the bwd half of a batch pair.
            VE partition bases must be 32-aligned: pairs give 32-row ops."""
            p0 = 2 * bp * 16
            nc.vector.tensor_copy(
                xg_tile[64 + p0:64 + p0 + 32, :].rearrange(
                    "p (g s) -> p g s", g=4),
                btmp[p0:p0 + 32, :].rearrange(
                    "p (g s) -> p g s", g=4)[:, :, ::-1])

        for cc in range(NCC):
            lo, hi = BOUNDS[cc], BOUNDS[cc + 1]
            CH = hi - lo
            # all loads were issued upfront; a gather waiting on drain sems
            # can no longer block any load issue in its engine FIFO.
            if cc == 4:
                emit_gathers(0)      # batch 0: drains done since cc=2
            if cc == 6:
                emit_gathers(1)
            if cc == 8:
                emit_gathers(2)
                emit_reverse(0)      # batches 0-1 btmp landed long ago
            tc0 = 30 + cc * 6
            nc.tensor.matmul(psscr[0:2, tc0:tc0 + 2],
                             lhsT=xt0_sb[:, lo:lo + 2],
                             rhs=xt0_sb[:, lo:lo + 2], start=True, stop=True)
            nc.tensor.matmul(psscr[0:2, tc0 + 2:tc0 + 4],
                             lhsT=xt1_sb[:, lo:lo + 2],
                             rhs=xt1_sb[:, lo:lo + 2], start=True, stop=True)
            nc.tensor.matmul(psscr[0:2, tc0 + 4:tc0 + 6],
                             lhsT=xt2_sb[:, lo:lo + 2],
                             rhs=xt2_sb[:, lo:lo + 2], start=True, stop=True)
            b = lo // T
            toff = lo % T
            ng = CH // 512
            pss = [psA.tile([8, 512], F32, tag="ps", name=f"ps{n}")
                   for n in range(ng)]
            for wtile, xtile, st_, sp_ in (
                    (w8a, xt0_sb, True, False),
                    (w8b, xt1_sb, False, False),
                    (w8c, xt2_sb, False, True)):
                for n in range(ng):
                    csl = slice(lo + n * 512, lo + n * 512 + 512)
                    nc.tensor.matmul(pss[n], lhsT=wtile, rhs=xtile[:, csl],
                                     start=st_, stop=sp_)
            # drain PSUM straight into xball (fp16 cast on copy),
            # alternating VE/ACT so the banks free fast enough for the PE.
            xc0 = b * XROW + W + toff
            for n in range(ng):
                dsl = slice(xc0 + n * 512, xc0 + (n + 1) * 512)
                if n % 2 == 0:
                    nc.vector.tensor_copy(xball[:, dsl], pss[n])
                else:
                    nc.scalar.activation(xball[:, dsl], pss[n], AF.Identity)

        emit_gathers(3, tail=True)   # batch 3: the only tail work
        emit_reverse(1)

        p1ctx.close()
        xbctx.close()
        scan2 = ExitStack()
        scanq = scan2.enter_context(tc.tile_pool(name="scanq", bufs=1))
        psB = scan2.enter_context(tc.tile_pool(name="psB", bufs=1,
                                               space="PSUM"))

        # ---- phase 3: fixed-point iterations ----
        wscr = psB.tile([8, 1024], F32, tag="wscr")
        sf = scanq.tile([128, 2 * N_ITER], F16, tag="sf")
        gbuf = scanq.tile([128, 4 * S], F32, tag="gbuf")
        St = scanq.tile([128, 3 * S], F32, tag="St")
        Gt = scanq.tile([128, S], F32, tag="Gt")
        mt = scanq.tile([128, S], F32, tag="mt")
        ct = scanq.tile([128, S], F32, tag="ct")
        tct = scanq.tile([128, S], F32, tag="tct")
        escr = singles.tile([1, 1], F32)
        for it in range(N_ITER):
            # per-gate STT -> ACT interleave: each activation issues as soon
            # as its gate's pre-activation is ready, overlapping VE and ACT.
            # Iteration 0 has h == 0, so ACT reads (fp16) xg_tile directly.
            for g, fn, dsl in ((3, AF.Tanh, -1), (0, AF.Sigmoid, 0),
                               (1, AF.Sigmoid, 1), (2, AF.Sigmoid, 2)):
                if it == 0:
                    src = xg_tile
                else:
                    src = gbuf
                    nc.vector.scalar_tensor_tensor(
                        out=gbuf[:, g * S:(g + 1) * S],
                        in0=h_st[:, 0:S],
                        scalar=whh_sb[:, g:g + 1],
                        in1=xg_tile[:, g * S:(g + 1) * S],
                        op0=ALU.mult, op1=ALU.add)
                if dsl < 0:
                    nc.scalar.activation(Gt, src[:, 3 * S:4 * S], AF.Tanh)
                else:
                    nc.scalar.activation(St[:, dsl * S:(dsl + 1) * S],
                                         src[:, g * S:(g + 1) * S], fn)
            nc.vector.tensor_mul(mt, St[:, 0:S], Gt)
            nc.vector.tensor_tensor_scan(
                out=ct, data0=St[:, S:2 * S], data1=mt, initial=0.0,
                op0=ALU.mult, op1=ALU.add)
            nc.scalar.activation(tct, ct, AF.Tanh)
            if it == N_ITER - 1:
                # prefetch the Exp ACT table set (~1.3us) off the critical
                # path.  Reading tct pins this AFTER the last tanh in the
                # schedule (a negone read let the scheduler hoist it before
                # the scan's sigmoids, forcing a second table load).
                nc.scalar.activation(escr, tct[0:1, 0:1], AF.Exp)
            # keep-warm: a few matmuls per iteration (fed by a tiny copy so
            # they never WAR back into scan state) hold the PE clock at 8/8.
            nc.vector.tensor_copy(sf[:, it * 2:it * 2 + 2], ct[:, 0:2])
            for sp in range(3):
                nc.tensor.matmul(
                    wscr[0:2, 0:512],
                    lhsT=sf[:, it * 2:it * 2 + 2],
                    rhs=xg_tile[:, 0:512], start=True, stop=True)
            nc.vector.tensor_mul(h_st[:, 1:S + 1], St[:, 2 * S:3 * S], tct)

        scan2.close()
        scanctx.close()
        p4ctx = ExitStack()
        late = p4ctx.enter_context(tc.tile_pool(name="late", bufs=1))
        p4ps = ExitStack()
        psC = p4ps.enter_context(tc.tile_pool(name="psC", bufs=1,
                                              space="PSUM"))

        # ---- phase 4: attention ----
        # bwd h alignment: row (d=1,b,k) col c holds h for t = k*L + S - c,
        # so one flip time-aligns it with the fwd rows.  The critical-chain
        # tiles live in `singles` — a fresh pool here would reuse the scan
        # pool's bytes and pick up WAR waits on the scan's last readers.
        h_rev = singles.tile([64, S + 1], F32)
        nc.vector.tensor_copy(h_rev, h_st[64:128, ::-1])
        hsum = singles.tile([64, L], F32)
        nc.vector.tensor_add(hsum, h_st[0:64, W + 1:S + 1], h_rev[:, 0:L])
        # logits = 0.5*hsum with hsum in (-2,2): exp(0.5*hsum - 1) is always
        # in [e^-2, 1], so no max-subtraction is needed for stability.
        exps = singles.tile([64, L], F32)
        s1 = singles.tile([64, 1], F32)
        nc.scalar.activation(exps, hsum, AF.Exp, bias=negone[:, :], scale=0.5,
                             accum_out=s1)
        ps_s = psC.tile([4, 1], F32)
        nc.tensor.matmul(ps_s, lhsT=sel_sb, rhs=s1, start=True, stop=True)
        r4 = singles.tile([4, 1], F32)
        nc.vector.reciprocal(r4, ps_s)
        ps_r = psC.tile([64, 1], F32, tag="psr")
        nc.tensor.matmul(ps_r, lhsT=selT_sb, rhs=r4, start=True, stop=True)
        wscr2 = psC.tile([2, 256], F32, tag="wscr2")
        for sp in range(3):
            nc.tensor.matmul(wscr2[0:2, 0:256],
                             lhsT=exps[:, sp * 2:sp * 2 + 2],
                             rhs=exps[:, :],
                             start=True, stop=True)
        att_r = late.tile([64, L], F16)
        nc.vector.tensor_scalar_mul(att_r, exps, ps_r[:, 0:1])
        # flatten att to token order on ONE partition so phase-5 matmuls
        # can use it as a base-partition-0 moving operand.  With b-major
        # rows, partition-major element order IS token order: one plain
        # SBUF->SBUF DMA.
        attT = late.tile([1, TOK], F16)
        nc.sync.dma_start(out=attT[0:1, :], in_=att_r)
        # touch attT + ones16 so phase-5 matmuls carry only their PSUM wait
        ps_t = psC.tile([2, 2], F32, tag="pst")
        nc.tensor.matmul(ps_t, lhsT=attT[0:1, 0:2], rhs=attT[0:1, 0:2],
                         start=True, stop=True)
        ps_t2 = psC.tile([2, 2], F32, tag="pst2")
        nc.tensor.matmul(ps_t2, lhsT=ones16[0:1, 0:2], rhs=ones16[0:1, 0:2],
                         start=True, stop=True)

        p4ps.close()
        p5ctx = ExitStack()
        opool0 = p5ctx.enter_context(tc.tile_pool(name="opool0", bufs=2))
        opool1 = p5ctx.enter_context(tc.tile_pool(name="opool1", bufs=2))
        opool2 = p5ctx.enter_context(tc.tile_pool(name="opool2", bufs=2))
        apool = p5ctx.enter_context(tc.tile_pool(name="apool", bufs=2))
        psP = p5ctx.enter_context(tc.tile_pool(name="psP", bufs=4,
                                               space="PSUM"))

        # ---- phase 5: out = x * att ----
        # att broadcast across partitions via PE outer product (ones^T @ att
        # slice) into [128,1024] PSUM tiles (4 bufs); ACT copies PSUM ->
        # fp16 SBUF; VE multiplies 4096-col tiles (all three row groups --
        # GpSimd TT measured ~4x slower and VE+GP concurrency degrades both
        # ~4x via SBUF port contention); fp16 writes on the two HWDGE
        # queues.  Device output is fp16; the host widens to fp32.
        # tt=0 runs in 2048-col halves so the first VE multiply starts
        # after 2 PSUM quarters instead of 4 (faster pipeline ramp).
        pieces = [(0, 2048), (2048, 2048)] + [(tt * 4096, 4096)
                                              for tt in range(1, 4)]
        for pi, (c0, CW) in enumerate(pieces):
            cols = slice(c0, c0 + CW)
            att16 = apool.tile([128, 4096], F16, tag="att16")
            a16 = att16[:, 0:CW]
            for q in range(CW // 1024):
                pa = psP.tile([128, 1024], F32, tag="pa")
                for j2 in range(2):
                    cj = c0 + q * 1024 + j2 * 512
                    nc.tensor.matmul(
                        pa[:, j2 * 512:(j2 + 1) * 512],
                        lhsT=ones16,
                        rhs=attT[0:1, cj:cj + 512],
                        start=True, stop=True)
                nc.scalar.activation(a16[:, q * 1024:(q + 1) * 1024],
                                     pa, AF.Identity)
            ob0 = opool0.tile([128, 4096], F16, tag="ob0")
            nc.vector.tensor_mul(ob0[:, 0:CW], xt0_sb[:, cols], a16)
            nc.sync.dma_start(out=outT[0:128, cols], in_=ob0[:, 0:CW])
            ob1 = opool1.tile([128, 4096], F16, tag="ob1")
            nc.vector.tensor_mul(ob1[:, 0:CW], xt1_sb[:, cols], a16)
            nc.scalar.dma_start(out=outT[128:256, cols], in_=ob1[:, 0:CW])
            ob2 = opool2.tile([44, 4096], F16, tag="ob2")
            nc.vector.tensor_mul(ob2[0:44, 0:CW], xt2_sb[0:44, cols],
                                 a16[0:44, :])
            if pi % 2 == 0:
                nc.sync.dma_start(out=outT[256:300, cols],
                                  in_=ob2[0:44, 0:CW])
            else:
                nc.scalar.dma_start(out=outT[256:300, cols],
                                    in_=ob2[0:44, 0:CW])
        p5ctx.close()
        p4ctx.close()

    return nc


_NC = None


def _get_nc():
    global _NC
    if _NC is None:
        _NC = _build_nc()
        _NC.finalize()
    return _NC


def _prep_core_inputs(x, w_ih_f, w_hh_f, b_ih_f, b_hh_f,
                      w_ih_b, w_hh_b, b_ih_b, b_hh_b):
    """Build the per-core input maps."""
    w8 = np.zeros((301, 8), np.float32)   # rows: e 0..299, 300 = bias
    whh = np.zeros((P, 4), np.float32)
    for d, (wi, wh, bi, bh) in enumerate(
            [(w_ih_f, w_hh_f, b_ih_f, b_hh_f),
             (w_ih_b, w_hh_b, b_ih_b, b_hh_b)]):
        for j, gp in enumerate(GATE_PERM):
            w8[0:300, d * 4 + j] = wi[gp, :]
            w8[300, d * 4 + j] = bi[gp] + bh[gp]
            whh[d * 64:(d + 1) * 64, j] = wh[gp, 0]
    w8 = w8.astype(np.float16)
    w8a = np.ascontiguousarray(w8[0:128])
    w8b = np.ascontiguousarray(w8[128:256])
    w8c = np.zeros((45, 8), np.float16)
    w8c[0:44] = w8[256:300]
    w8c[44] = w8[300]
    sel = np.zeros((64, 4), np.float32)
    for r in range(64):
        sel[r, r // 16] = 1.0
    selT = np.ascontiguousarray(sel.T)

    maps = []
    for c in range(NCORES):
        xs = x[c * BL:(c + 1) * BL]                       # [4, T, E]
        xTc = xs.transpose(2, 0, 1).reshape(E, TOK).astype(np.float16)
        xt2 = np.ones((45, TOK), np.float16)
        xt2[0:44] = xTc[256:300]
        maps.append({"xt0": np.ascontiguousarray(xTc[0:128]),
                     "xt1": np.ascontiguousarray(xTc[128:256]),
                     "xt2": xt2,
                     "w8a": w8a, "w8b": w8b, "w8c": w8c,
                     "whh": whh, "sel": sel, "selT": selT})
    return maps


def _run(inputs, trace=False, tmpdir=None):
    nc = _get_nc()
    maps = _prep_core_inputs(**inputs)
    res = run_bass_kernel_spmd(nc, maps, list(range(NCORES)), trace=trace,
                               tmpdir=tmpdir)
    outs = []
    for c in range(NCORES):
        oT = res.results[c]["outT"].astype(np.float32)    # [E, TOK] fp16
        outs.append(oT.reshape(E, BL, T).transpose(1, 2, 0))
    return np.concatenate(outs, axis=0), res


def kernel(**inputs):
    out, _ = _run(inputs, trace=False)
    return out
